# revision 27
# baseline (speedup 1.0000x reference)
"""Trainium2 Bass kernel for policy-masked attention (nn_Attention_5007931867377).

Reference computation (per batch b):
    qkv = x @ w_qkv.T ; split into q,k,v heads [H=6, N=1568, D=64]
    s   = (q @ k.T) * D**-0.5
    mask[m] visibility per key + diagonal always kept
    e   = exp(s - max) * mask ; attn = (e + EPS/N)/(sum e + EPS)
    out = (attn @ v) concat heads @ w_proj.T + b_proj

Strategy: pure data parallel, one batch element per NeuronCore (8 cores).
Per-core dataflow (everything transposed so softmax reductions are on the
free axis and e^T feeds the V-matmul without on-chip transposes):
  - host passes x^T (bf16); on chip: Q^T,K^T in [c_out, n] layout, V in [n, d]
  - scores^T[m, q] = K^T.T @ Q^T  (PSUM, per 128-key chunk x query piece)
  - key-mask folded into exp as per-partition bias (-30 * (1-p));
    diagonal-keep folded into scores via +240*(1-p) diag matmul (exp scale
    0.125 makes that +30, cancelling the -30 bias exactly on the diagonal)
  - e^T = exp(0.125*s + bias) ACT -> SBUF (bf16)
  - V augmented with a ones column: outT_aug[65, q] = V_aug.T @ e^T gives
    attention output rows 0..63 and the softmax denominator in row 64
  - normalize with reciprocal_approx_fast + gpsimd partition_broadcast
  - proj: y[n, :] = oT.T @ w_proj.T (+ bias via K=1 ones matmul), DMA out.

Schedule: piece-scoped head passes (h, query-piece) over the KC kept key
chunks, software-pipelined so PE and ACT overlap:
  - exp is issued WIDE (768/800-col pieces, each 2 PSUM banks) to amortize
    the ~200-cycle ACTIVATE init; per-head band (diag-only) blocks batch
    into one [128, NB*128] tile with a single exp (dfix written first per
    bank so the diag score matmuls accumulate onto it).
  - scores are dual-row-tiled: a half-swapped mirror (qm) of Q^T/K^T lets
    chunk pairs run concurrently on PE row halves T0/T8.
  - PSUM: 1 out accumulator (2 banks, bufs=1) + sc pool (2 banks x bufs=3);
    the accumulator is released by one DVE copy to SBUF staging and the
    norm chain (recip + gpsimd broadcast + mul) runs off the critical path.
  - qkv cc1/4 + cc2/5 units and piece-0 projections interleave as PE
    fillers inside later passes; out DMA splits across two queue groups.
All matmuls bf16 (HAM-warm PE).  Max-subtraction is dropped: scores ~
N(0,1), |s| < ~6.
"""

import sys

if "/opt/trn_rl_repo" not in sys.path:
    sys.path.insert(0, "/opt/trn_rl_repo")

import numpy as np

B, N, C, H = 8, 1568, 384, 6
D = C // H  # 64
SCALE = D ** -0.5  # 0.125
EPS = 1e-6
NEG = -30.0  # masked-key exp bias (exp(-30) ~ 9e-14, way below EPS/N)
DIAGV = -NEG / SCALE  # 240.0 added to diagonal scores, cancels bias exactly

P = 128
NCH = (N + P - 1) // P  # 13 key/token chunks (12 x 128 + 1 x 32)
CHS = [min(P, N - i * P) for i in range(NCH)]
# query pieces: chunk-aligned (multiples of 128 so band blocks nest), each
# <= 2 PSUM banks; matmuls subsplit at 512 (bank bounds) inside a piece
QPIECES = [(0, 768), (768, 800)]
SLOTW = 1024  # PSUM slot width (2 banks f32)
NPIECE = len(QPIECES)


def _subsplit(qo, qw, step=512):
    return [(qo + o, min(step, qw - o)) for o in range(0, qw, step)]


_CACHE = {}


def _build_nc(KC, BSTART, HASB):
    """Build the SPMD program.

    Tokens are permuted host-side so policy-kept keys come first. KC = number
    of 128-key chunks holding any kept key (full scores+softmax+V path);
    chunks BSTART..NCH-1 contain dropped keys, which are visible only to
    their own query (diagonal) — handled by small diagonal-block tasks
    (constant -30 exp bias + dfix diag matmul restores exactly the
    (1-p) diagonal entries), batched per head into one wide exp.
    """
    import concourse.tile as tile
    from concourse import bacc, mybir

    dt = mybir.dt
    f32 = dt.float32
    bf16 = dt.bfloat16
    AF = mybir.ActivationFunctionType

    NB = NCH - BSTART  # number of band (diag-only) chunks

    nc = bacc.Bacc()

    xT_d = nc.declare_dram_parameter("xT", [C, N], bf16, isOutput=False)
    wqkvT_d = nc.declare_dram_parameter("wqkvT", [C, 3 * C], bf16, isOutput=False)
    wprojT_d = nc.declare_dram_parameter("wprojT", [C, C], bf16, isOutput=False)
    bias_d = nc.declare_dram_parameter("bias_exp", [P, NCH], f32, isOutput=False)
    dfix_d = nc.declare_dram_parameter("dfix", [P, NCH, P], bf16, isOutput=False)
    ident_d = nc.declare_dram_parameter("ident", [P, P], bf16, isOutput=False)
    bvec_d = nc.declare_dram_parameter("bvec", [1, C], bf16, isOutput=False)
    out_d = nc.declare_dram_parameter("out", [N, C], f32, isOutput=True)

    with tile.TileContext(nc, pool_alloc_mode="queue") as tc:
        with (
            tc.tile_pool(name="persist", bufs=1) as pp,
            tc.tile_pool(name="work", bufs=6) as wp,
        ):
            # ---- persistent SBUF tensors ----
            xt = pp.tile([P, 3, N], bf16, tag="xt")  # x^T chunks (c rows)
            wqkv = pp.tile([P, 3, 3 * C], bf16, tag="wqkv")
            wproj = pp.tile([P, 3, C], bf16, tag="wproj")
            qk = pp.tile([P, 6, N], bf16, tag="qk")  # Q^T (0..2), K^T (3..5)
            # half-swapped mirror of qk: qm[0:64,c]=qk[64:128,c] and vice
            # versa, so both PE row-halves can stream any head (dual-row
            # tiled score matmuls run two key chunks concurrently)
            qm = pp.tile([P, 6, N], bf16, tag="qm")
            vaug = pp.tile([P, NCH, H, D + 1], bf16, tag="vaug")
            ot = pp.tile([P, 3, N], bf16, tag="ot")  # normalized attn out ^T
            bias = pp.tile([P, NCH], f32, tag="bias")
            dfix = pp.tile([P, NCH, P], bf16, tag="dfix")
            ident = pp.tile([P, P], bf16, tag="ident")
            bvec = pp.tile([1, C], bf16, tag="bvec")
            ones = pp.tile([1, P], bf16, tag="ones")
            negb = pp.tile([P, 1], f32, tag="negb")  # band exp bias (-30)

            # warm-up constants first: the PE dummy matmuls and the ACT
            # table load must not wait behind the input DMA queue
            nc.vector.memset(ones[:, :], 1.0)
            nc.vector.memset(negb[:, :], NEG)
            warm = pp.tile([1, 1], f32, tag="warm")
            nc.scalar.activation(warm[:, :], negb[0:1, :], AF.Exp)
            wrow = pp.tile([1, 512], bf16, tag="wrow")
            nc.vector.memset(wrow[:, :], 0.0)

            # DMA order follows the upfront block's dependency chain: the
            # first qkv unit needs xt[:, :, 0:512] + w_qkv cc0 columns.
            xr = xT_d[:].rearrange("(a p) n -> p a n", p=P)
            qr_ = wqkvT_d[:].rearrange("(a p) n -> p a n", p=P)
            pr = wprojT_d[:].rearrange("(a p) n -> p a n", p=P)
            for c in range(3):
                nc.sync.dma_start(xt[:, c, :512], xr[:, c, :512])
            for o, w in ((0, P), (3 * P, P)):  # w_qkv cc0, cc3 columns
                for c in range(3):
                    nc.sync.dma_start(
                        wqkv[:, c, o : o + w], qr_[:, c, o : o + w]
                    )
            for c in range(3):
                nc.sync.dma_start(xt[:, c, 512:1024], xr[:, c, 512:1024])
            nc.sync.dma_start(bias[:, :], bias_d[:])
            for c in range(3):  # V columns
                nc.sync.dma_start(
                    wqkv[:, c, 2 * C : 3 * C], qr_[:, c, 2 * C : 3 * C]
                )
            nc.sync.dma_start(ident[:, :], ident_d[:])
            for c in range(3):
                nc.sync.dma_start(xt[:, c, 1024:], xr[:, c, 1024:])
            db = min(8, NCH)
            nc.sync.dma_start(dfix[:, :db, :], dfix_d[:][:, :db, :])
            for o, w in ((P, 2 * P), (4 * P, 2 * P)):  # cc1/2, cc4/5 cols
                for c in range(3):
                    nc.sync.dma_start(
                        wqkv[:, c, o : o + w], qr_[:, c, o : o + w]
                    )
            nc.sync.dma_start(dfix[:, db:, :], dfix_d[:][:, db:, :])
            for c in range(3):
                nc.sync.dma_start(wproj[:, c, :], pr[:, c, :])
            nc.sync.dma_start(bvec[:, :], bvec_d[:])

            # ---- qkv / V / mirror unit helpers ----
            # All projection work is emitted as PE filler units inside the
            # attention pass stream (PSUM tag "sc" rotation), so the exp
            # pipeline starts as soon as the first operands land.
            def _mirror(c, lo, hi):
                nc.gpsimd.dma_start(qm[64:128, c, lo:hi], qk[0:64, c, lo:hi])
                nc.gpsimd.dma_start(qm[0:64, c, lo:hi], qk[64:128, c, lo:hi])

            # ---- phase 2: attention ----
            # Piece-scoped head passes: for each (head, query-piece) the KC
            # kept chunks stream scores->exp->vmm with sc triple-buffered
            # (6 banks) while ONE out accumulator (2 banks) persists.  The
            # accumulator is released by a fast DVE copy to SBUF staging;
            # the normalization chain runs off the critical path.
            with (
                tc.tile_pool(name="outps", bufs=1, space="PSUM") as ops,
                tc.tile_pool(name="scps", bufs=3, space="PSUM") as sps,
            ):
                # last-writer bookkeeping per (piece, sub-region)
                last_band = {}
                kept_is_last = {}
                for pi, (qo, qw) in enumerate(QPIECES):
                    for (so, sw) in _subsplit(qo, qw):
                        js = [j for j in range(BSTART, NCH)
                              if so <= j * P and j * P + CHS[j] <= so + sw]
                        last_band[(pi, so)] = max(js) if js else None
                        kept_is_last[(pi, so)] = not js

                passes = [(h, pi) for pi in range(NPIECE) for h in range(H)]
                scmap = {}

                def _qkv_unit(cc, qo, qw):
                    def emit():
                        ps = sps.tile([P, 512], f32, tag="sc",
                                      name=f"qp{cc}_{qo}")
                        for c in range(3):
                            nc.tensor.matmul(
                                ps[:, :qw],
                                wqkv[:, c, cc * P : (cc + 1) * P],
                                xt[:, c, qo : qo + qw],
                                start=(c == 0),
                                stop=(c == 2),
                            )
                        nc.vector.tensor_copy(
                            qk[:, cc, qo : qo + qw], ps[:, :qw]
                        )
                    return emit

                def _v_unit(i):
                    def emit():
                        m = CHS[i]
                        ps = sps.tile([P, C], f32, tag="sc", name=f"v{i}")
                        for c in range(3):
                            nc.tensor.matmul(
                                ps[:m, :],
                                xt[:, c, i * P : i * P + m],
                                wqkv[:, c, 2 * C : 3 * C],
                                start=(c == 0),
                                stop=(c == 2),
                            )
                        nc.vector.tensor_copy(
                            vaug[:m, i, :, 0:D],
                            ps[:m, :].rearrange("p (h d) -> p h d", h=H),
                        )
                        nc.vector.memset(vaug[:m, i, :, D : D + 1], 1.0)
                    return emit

                def emit_scores_g(h, pi, i, half=None):
                    # half 0 -> PE rows 0-63 (tile T0), half 1 -> rows
                    # 64-127 (T8); the head's natural half uses qk, the
                    # other uses the qm mirror.  Adjacent T0/T8 matmuls
                    # execute concurrently (independent row groups).
                    kc, qc = 3 + h // 2, h // 2
                    if half is None:
                        half = h % 2
                    src_t = qk if half == (h % 2) else qm
                    r = half * D
                    qo, qw = QPIECES[pi]
                    m = CHS[i]
                    sc = sps.tile([P, SLOTW], f32, tag="sc",
                                  name=f"sc{h}{pi}_{i}")
                    subs = _subsplit(qo, qw)
                    for si, (so, sw) in enumerate(subs):
                        nc.tensor.matmul(
                            sc[:m, so - qo : so - qo + sw],
                            src_t[r : r + D, kc, i * P : i * P + m],
                            src_t[r : r + D, qc, so : so + sw],
                            start=True,
                            stop=si == len(subs) - 1,
                        )
                    scmap[(h, pi, i)] = sc

                # deferred fillers keyed by (pass_idx, chunk_i): remaining
                # qkv channel units, V chunks, and later-wave channels
                fillers = {}
                prefill = {}

                def _add(key, fn):
                    fillers.setdefault(key, []).append(fn)

                # Passes 0-1 (h0/h1, piece 0) run during the input DMA
                # stream with natural-half single scores (no mirror dep);
                # remaining qkv channels, V chunks and mirrors are fillers
                # ordered to match DMA arrival.  Q channels mirror both
                # column ranges; K channels only [0,1024) (kept keys).
                KCm = KC - 1

                def _ki(i):
                    return min(i, KCm)

                prefill[(0, 0)] = [_v_unit(0)]
                _add((0, _ki(0)), _v_unit(1))
                _add((0, _ki(0)), _v_unit(2))
                _add((0, _ki(1)), _v_unit(3))
                _add((0, _ki(1)), _qkv_unit(3, 512, 512))
                _add((0, _ki(2)), _v_unit(4))
                _add((0, _ki(2)), _v_unit(5))
                _add((0, _ki(3)), _v_unit(6))
                _add((0, _ki(3)), _v_unit(7))
                _add((0, _ki(4)), _qkv_unit(0, 1024, 512))
                _add((0, _ki(4)), _qkv_unit(3, 1024, 512))
                _add((0, _ki(5)), _v_unit(8))
                _add((0, _ki(5)), _v_unit(9))
                _add((0, _ki(5)), _qkv_unit(0, 1536, N - 1536))
                _add((0, _ki(6)), _v_unit(10))
                _add((0, _ki(6)), _v_unit(11))
                _add((0, _ki(6)), _qkv_unit(3, 1536, N - 1536))
                _add((1, _ki(0)), _v_unit(12))
                _add((1, _ki(0)), lambda: _mirror(3, 0, 1024))
                _add((1, _ki(0)), lambda: _mirror(0, 0, 1024))
                _add((1, _ki(0)), lambda: _mirror(0, 1024, N))
                for wave, (ccq, cck) in enumerate(((1, 4), (2, 5))):
                    # core units feed the next pair of piece-0 passes; the
                    # column-1024+ tails are only needed by piece-1 passes
                    # (>= idx 6), so they spread into later passes instead
                    # of bunching (which starved ACT at pass boundaries)
                    core = [
                        _qkv_unit(ccq, 0, 512),
                        _qkv_unit(ccq, 512, 512),
                        lambda c=ccq: _mirror(c, 0, 1024),
                        _qkv_unit(cck, 0, 512),
                        _qkv_unit(cck, 512, 512),
                        lambda c=cck: _mirror(c, 0, 1024),
                    ]
                    tail = [
                        _qkv_unit(ccq, 1024, 512),
                        _qkv_unit(ccq, 1536, N - 1536),
                        lambda c=ccq: _mirror(c, 1024, N),
                        _qkv_unit(cck, 1024, 512),
                        _qkv_unit(cck, 1536, N - 1536),
                    ]
                    for u, unit in enumerate(core):
                        _add((1 + wave, _ki(1 + min(u, KC - 2))), unit)
                    for u, unit in enumerate(tail):
                        _add((3 + wave, _ki(1 + min(u, KC - 2))), unit)

                # piece-0 projections become PE fillers spread across the
                # piece-1 passes (ot piece 0 is complete after pass H-1)
                p0_chunks = [j for j in range(NCH)
                             if j * P + CHS[j] <= QPIECES[0][1]]
                for k, j in enumerate(p0_chunks):
                    key = (H + min(k, H - 1), 2)
                    fillers.setdefault(key, []).append(
                        lambda j=j: emit_proj(j, use_dve=(j % 2 == 0)))

                def emit_pass(idx):
                    h, pi = passes[idx]
                    kc, kr = 3 + h // 2, (h % 2) * D
                    qc, qr = h // 2, (h % 2) * D
                    qo, qw = QPIECES[pi]
                    subs = _subsplit(qo, qw)
                    outs = ops.tile([D + 1, SLOTW], f32, tag="outT",
                                    name=f"o{h}_{pi}")
                    ets = {}
                    # chunk-pair schedule: after exp(i) emit scores for the
                    # next pair (dual-row tiled, both halves) or single.
                    # Passes 0-1 run before the qm mirrors exist -> singles.
                    sched = {}
                    if idx < 2:
                        for a in range(1, KC):
                            sched[a - 1] = (a,)
                    else:
                        a = 1
                        while a < KC:
                            if a + 1 < KC:
                                sched[a - 1] = (a, a + 1)
                                a += 2
                            else:
                                sched[a - 1] = (a,)
                                a += 1

                    for i in range(KC):
                        m = CHS[i]
                        et = wp.tile([P, SLOTW], bf16, tag="et",
                                     name=f"et{h}{pi}_{i}")
                        nc.scalar.activation(
                            et[:m, :qw],
                            scmap.pop((h, pi, i))[:m, :qw],
                            AF.Exp,
                            bias=bias[:m, i : i + 1],
                            scale=SCALE,
                        )
                        ets[i] = et
                        for k, nx in enumerate(sched.get(i, ())):
                            emit_scores_g(h, pi, nx,
                                          half=(k if len(sched[i]) == 2
                                                else None))
                        for unit in prefill.get((idx, i), []):
                            unit()
                        for (so, sw) in subs:
                            nc.tensor.matmul(
                                outs[:, so - qo : so - qo + sw],
                                vaug[:m, i, h, :],
                                ets[i][:m, so - qo : so - qo + sw],
                                start=(i == 0),
                                stop=(i == KC - 1 and kept_is_last[(pi, so)]),
                            )
                        del ets[i]
                        for unit in fillers.get((idx, i), []):
                            unit()

                    # band (diag-only) blocks of this piece: dfix written
                    # first per 512-bank (start=True sets has_written for
                    # the whole region), then the diag score matmuls
                    # accumulate, then ONE wide exp per <=8 blocks.
                    bj = [j for j in range(BSTART, NCH)
                          if qo <= j * P and j * P + CHS[j] <= qo + qw]
                    bscr = []
                    for g0 in range(0, len(bj), 8):
                        grp8 = bj[g0 : g0 + 8]
                        scb = sps.tile([P, SLOTW], f32, tag="sc",
                                       name=f"sb{h}{pi}_{g0}")
                        for b0 in range(0, len(grp8), 4):
                            grp = grp8[b0 : b0 + 4]
                            off = b0 * P
                            nc.tensor.matmul(
                                scb[:, off : off + len(grp) * P],
                                ident[:, :],
                                dfix[:, grp[0] : grp[0] + len(grp), :],
                                start=True,
                                stop=False,
                            )
                            # natural half only: concurrent row tiles must
                            # not write the same PSUM bank
                            r = (h % 2) * D
                            for bi, j in enumerate(grp):
                                m = CHS[j]
                                nc.tensor.matmul(
                                    scb[:m, off + bi * P : off + bi * P + m],
                                    qk[r : r + D, kc, j * P : j * P + m],
                                    qk[r : r + D, qc, j * P : j * P + m],
                                    start=False,
                                    stop=(bi == len(grp) - 1),
                                )
                        bscr.append((grp8, scb))

                    # pre-emit the next pass's first scores so the PE can
                    # run them while this pass's band exp drains
                    if idx + 1 < len(passes):
                        nh, npi = passes[idx + 1]
                        emit_scores_g(nh, npi, 0)

                    for grp8, scb in bscr:
                        etb = wp.tile([P, SLOTW], bf16, tag="et",
                                      name=f"eb{h}{pi}_{grp8[0]}")
                        wtot = len(grp8) * P
                        nc.scalar.activation(
                            etb[:, :wtot],
                            scb[:, :wtot],
                            AF.Exp,
                            bias=negb[:, :],
                            scale=SCALE,
                        )
                        for bi, j in enumerate(grp8):
                            m = CHS[j]
                            for (so, sw) in subs:
                                if so <= j * P and j * P + CHS[j] <= so + sw:
                                    nc.tensor.matmul(
                                        outs[:, j * P - qo : j * P - qo + m],
                                        vaug[:m, j, h, :],
                                        etb[:m, bi * P : bi * P + m],
                                        start=False,
                                        stop=last_band[(pi, so)] == j,
                                    )

                    # fast PSUM release: one DVE copy to SBUF staging frees
                    # the accumulator; the normalization chain (reciprocal,
                    # gpsimd broadcast, multiply) runs off the critical path
                    stg = wp.tile([D + 1, SLOTW], f32, tag="stg", bufs=3,
                                  name=f"st{h}{pi}")
                    nc.vector.tensor_copy(stg[:, :qw], outs[:, :qw])
                    srow = wp.tile([1, SLOTW], f32, tag="srow", bufs=3,
                                   name=f"sr{h}{pi}")
                    nc.vector.tensor_copy(srow[:, :qw], stg[D : D + 1, :qw])
                    rinv = wp.tile([1, SLOTW], f32, tag="rinv", bufs=3,
                                   name=f"ri{h}{pi}")
                    nc.vector.reciprocal_approx_fast(
                        rinv[:, :qw], srow[:, :qw]
                    )
                    rbr = wp.tile([D, SLOTW], f32, tag="rbr", bufs=3,
                                  name=f"rr{h}{pi}")
                    nc.gpsimd.partition_broadcast(rbr[:, :qw], rinv[:, :qw])
                    nc.vector.tensor_mul(
                        ot[qr : qr + D, qc, qo : qo + qw],
                        stg[0:D, :qw],
                        rbr[:, :qw],
                    )

                    if h == H - 1 and pi == NPIECE - 1:
                        for j in range(NCH):
                            if qo <= j * P and j * P + CHS[j] <= qo + qw:
                                emit_proj(j, use_dve=(j % 2 == 0))

                def emit_proj(j, use_dve):
                    m = CHS[j]
                    yp = sps.tile([P, SLOTW], f32, tag="sc", name=f"yp{j}")
                    for c in range(3):
                        nc.tensor.matmul(
                            yp[:m, :C],
                            ot[:, c, j * P : j * P + m],
                            wproj[:, c, :],
                            start=(c == 0),
                            stop=(c == 2 and not HASB),
                        )
                    if HASB:
                        nc.tensor.matmul(
                            yp[:m, :C],
                            ones[:, :m],
                            bvec[:, :],
                            start=False,
                            stop=True,
                        )
                    ys = wp.tile([P, C], f32, tag="ys", name=f"ys{j}")
                    if use_dve:
                        nc.vector.tensor_copy(ys[:m, :], yp[:m, :C])
                    else:
                        nc.scalar.copy(ys[:m, :], yp[:m, :C])
                    # split per chunk across two hw queues (per-queue bw is
                    # ~23 GB/s; one 196KB chunk would be an ~8us drain) and
                    # dispatch from compute engines to spare the sync queue
                    hm = (m + 1) // 2
                    nc.gpsimd.dma_start(
                        out_d[j * P : j * P + hm, :], ys[0:hm, :]
                    )
                    nc.sync.dma_start(
                        out_d[j * P + hm : j * P + m, :], ys[hm:m, :]
                    )

                # prepass: the minimal qkv units for (h0, piece0) chunk 0
                # (the first real matmuls double as the HAM warm-up ramp)
                _qkv_unit(0, 0, 512)()
                _qkv_unit(3, 0, 512)()
                _qkv_unit(0, 512, 512)()
                emit_scores_g(*passes[0], 0)
                for idx in range(len(passes)):
                    emit_pass(idx)

    nc.finalize()
    return nc


def _prep_core_inputs(x_b, p_b, wqkvT, wprojT, bvec, ident):
    """Permute tokens kept-keys-first; build exp-bias and diag-fix tensors.
    Returns (in_map, perm)."""
    import ml_dtypes

    bf16 = ml_dtypes.bfloat16
    perm = np.argsort(-p_b, kind="stable")
    xT = np.ascontiguousarray(x_b[perm].T).astype(bf16)
    p_perm = p_b[perm].astype(np.float32)
    pad = NCH * P - N
    p_pad = np.concatenate([p_perm, np.zeros(pad, np.float32)])
    # bias_exp[r, i] = -30 * (1 - p[i*128 + r]) per key chunk
    bias = (NEG * (1.0 - p_pad)).reshape(NCH, P).T.copy()
    # dfix[:, i, :] = diag(240 * (1 - p_chunk_i)) as bf16
    dfix = np.zeros((P, NCH, P), np.float32)
    for i in range(NCH):
        chunk = p_pad[i * P : (i + 1) * P]
        np.fill_diagonal(dfix[:, i, :], DIAGV * (1.0 - chunk))
    return {
        "xT": xT,
        "wqkvT": wqkvT,
        "wprojT": wprojT,
        "bias_exp": np.ascontiguousarray(bias),
        "dfix": dfix.astype(bf16),
        "ident": ident,
        "bvec": bvec,
    }, perm


def _install_ntff_hook():
    """The container's antenv package lacks axon_hooks; recreate the NTFF
    profile hook (mirrors trn_agent_boot) so trace=True yields exec_time."""
    import types
    import ctypes
    import contextlib

    if "antenv.axon_hooks" in sys.modules:
        return
    so_path = "/opt/axon/libaxon_pjrt.so"
    mod = types.ModuleType("antenv.axon_hooks")
    state = {"hook": None}
    mod.set_axon_ntff_profile_hook = lambda h: state.__setitem__("hook", h)
    mod.get_axon_ntff_profile_hook = lambda: state["hook"]
    sys.modules["antenv.axon_hooks"] = mod

    try:
        lib = ctypes.CDLL(so_path)
    except OSError:
        return
    if not hasattr(lib, "axon_start_nrt_profile"):
        return
    lib.axon_start_nrt_profile.argtypes = [
        ctypes.POINTER(ctypes.c_int64),
        ctypes.c_size_t,
    ]
    lib.axon_start_nrt_profile.restype = ctypes.c_int64
    lib.axon_stop_nrt_profile.argtypes = [ctypes.c_char_p]
    lib.axon_stop_nrt_profile.restype = ctypes.c_int64

    @contextlib.contextmanager
    def _hook(output_dir, device_ids):
        import jax

        jax.devices()
        if device_ids:
            ids = (ctypes.c_int64 * len(device_ids))(*device_ids)
            rc = lib.axon_start_nrt_profile(ids, len(device_ids))
        else:
            rc = lib.axon_start_nrt_profile(None, 0)
        if rc != 0:
            raise RuntimeError(f"axon_start_nrt_profile rc={rc}")
        try:
            yield
        finally:
            n = lib.axon_stop_nrt_profile(str(output_dir).encode())
            print(f"profile: {n} file(s) written to {output_dir}", file=sys.stderr)

    state["hook"] = _hook


def kernel(x, vis_tube, w_qkv, w_proj, b_proj, _trace=False):
    from concourse.bass_utils import run_bass_kernel_spmd

    import ml_dtypes

    if _trace:
        _install_ntff_hook()

    bf16 = ml_dtypes.bfloat16
    x = np.asarray(x, np.float32)
    p = np.asarray(vis_tube, np.float32)[:, :, 0]
    keeps = (p > 0.5).sum(axis=1)  # kept keys per batch
    KC = max(1, int(-(-keeps.max() // P)))  # chunks containing kept keys
    BSTART = int(keeps.min() // P)  # first chunk containing a dropped key

    HASB = bool(np.any(np.asarray(b_proj)))
    key = (KC, BSTART, HASB)
    if _CACHE.get("key") != key:
        _CACHE["nc"] = _build_nc(KC, BSTART, HASB)
        _CACHE["key"] = key
    nc = _CACHE["nc"]

    wqkvT = np.ascontiguousarray(np.asarray(w_qkv).T).astype(bf16)
    wprojT = np.ascontiguousarray(np.asarray(w_proj).T).astype(bf16)
    bvec = np.asarray(b_proj).reshape(1, C).astype(np.float32).astype(bf16)
    ident = np.eye(P, dtype=np.float32).astype(bf16)
    in_maps, perms = [], []
    for b in range(B):
        im, perm = _prep_core_inputs(x[b], p[b], wqkvT, wprojT, bvec, ident)
        in_maps.append(im)
        perms.append(perm)
    res = run_bass_kernel_spmd(nc, in_maps, core_ids=list(range(B)), trace=_trace)
    out = np.empty((B, N, C), np.float32)
    for b in range(B):
        out[b][perms[b]] = np.asarray(res.results[b]["out"], np.float32)
    if _trace:
        _CACHE["last_result"] = res
    return out


# revision 29
# speedup vs baseline: 1.0117x; 1.0117x over previous
"""Trainium2 Bass kernel for policy-masked attention (nn_Attention_5007931867377).

Reference computation (per batch b):
    qkv = x @ w_qkv.T ; split into q,k,v heads [H=6, N=1568, D=64]
    s   = (q @ k.T) * D**-0.5
    mask[m] visibility per key + diagonal always kept
    e   = exp(s - max) * mask ; attn = (e + EPS/N)/(sum e + EPS)
    out = (attn @ v) concat heads @ w_proj.T + b_proj

Strategy: pure data parallel, one batch element per NeuronCore (8 cores).
Per-core dataflow (everything transposed so softmax reductions are on the
free axis and e^T feeds the V-matmul without on-chip transposes):
  - host passes x^T (bf16); on chip: Q^T,K^T in [c_out, n] layout, V in [n, d]
  - scores^T[m, q] = K^T.T @ Q^T  (PSUM, per 128-key chunk x query piece)
  - key-mask folded into exp as per-partition bias (-30 * (1-p));
    diagonal-keep folded into scores via +240*(1-p) diag matmul (exp scale
    0.125 makes that +30, cancelling the -30 bias exactly on the diagonal)
  - e^T = exp(0.125*s + bias) ACT -> SBUF (bf16)
  - V augmented with a ones column: outT_aug[65, q] = V_aug.T @ e^T gives
    attention output rows 0..63 and the softmax denominator in row 64
  - normalize with reciprocal_approx_fast + gpsimd partition_broadcast
  - proj: y[n, :] = oT.T @ w_proj.T (+ bias via K=1 ones matmul), DMA out.

Schedule: piece-scoped head passes (h, query-piece) over the KC kept key
chunks, software-pipelined so PE and ACT overlap:
  - exp is issued WIDE (768/800-col pieces, each 2 PSUM banks) to amortize
    the ~200-cycle ACTIVATE init; per-head band (diag-only) blocks batch
    into one [128, NB*128] tile with a single exp (dfix written first per
    bank so the diag score matmuls accumulate onto it).
  - scores are dual-row-tiled: a half-swapped mirror (qm) of Q^T/K^T lets
    chunk pairs run concurrently on PE row halves T0/T8.
  - PSUM: 1 out accumulator (2 banks, bufs=1) + sc pool (2 banks x bufs=3);
    the accumulator is released by one DVE copy to SBUF staging and the
    norm chain (recip + gpsimd broadcast + mul) runs off the critical path.
  - qkv cc1/4 + cc2/5 units and piece-0 projections interleave as PE
    fillers inside later passes; out DMA splits across two queue groups.
All matmuls bf16 (HAM-warm PE).  Max-subtraction is dropped: scores ~
N(0,1), |s| < ~6.
"""

import sys

if "/opt/trn_rl_repo" not in sys.path:
    sys.path.insert(0, "/opt/trn_rl_repo")

import numpy as np

B, N, C, H = 8, 1568, 384, 6
D = C // H  # 64
SCALE = D ** -0.5  # 0.125
EPS = 1e-6
NEG = -30.0  # masked-key exp bias (exp(-30) ~ 9e-14, way below EPS/N)
DIAGV = -NEG / SCALE  # 240.0 added to diagonal scores, cancels bias exactly

P = 128
NCH = (N + P - 1) // P  # 13 key/token chunks (12 x 128 + 1 x 32)
CHS = [min(P, N - i * P) for i in range(NCH)]
# query pieces: chunk-aligned (multiples of 128 so band blocks nest), each
# <= 2 PSUM banks; matmuls subsplit at 512 (bank bounds) inside a piece
QPIECES = [(0, 768), (768, 800)]
SLOTW = 1024  # PSUM slot width (2 banks f32)
NPIECE = len(QPIECES)


def _subsplit(qo, qw, step=512):
    return [(qo + o, min(step, qw - o)) for o in range(0, qw, step)]


_CACHE = {}


def _build_nc(KC, BSTART, HASB):
    """Build the SPMD program.

    Tokens are permuted host-side so policy-kept keys come first. KC = number
    of 128-key chunks holding any kept key (full scores+softmax+V path);
    chunks BSTART..NCH-1 contain dropped keys, which are visible only to
    their own query (diagonal) — handled by small diagonal-block tasks
    (constant -30 exp bias + dfix diag matmul restores exactly the
    (1-p) diagonal entries), batched per head into one wide exp.
    """
    import concourse.tile as tile
    from concourse import bacc, mybir

    dt = mybir.dt
    f32 = dt.float32
    bf16 = dt.bfloat16
    AF = mybir.ActivationFunctionType

    NB = NCH - BSTART  # number of band (diag-only) chunks

    nc = bacc.Bacc()

    xT_d = nc.declare_dram_parameter("xT", [C, N], bf16, isOutput=False)
    wqkvT_d = nc.declare_dram_parameter("wqkvT", [C, 3 * C], bf16, isOutput=False)
    wprojT_d = nc.declare_dram_parameter("wprojT", [C, C], bf16, isOutput=False)
    bias_d = nc.declare_dram_parameter("bias_exp", [P, NCH], f32, isOutput=False)
    dfix_d = nc.declare_dram_parameter("dfix", [P, NCH, P], bf16, isOutput=False)
    ident_d = nc.declare_dram_parameter("ident", [P, P], bf16, isOutput=False)
    bvec_d = nc.declare_dram_parameter("bvec", [1, C], bf16, isOutput=False)
    out_d = nc.declare_dram_parameter("out", [N, C], f32, isOutput=True)

    with tile.TileContext(nc, pool_alloc_mode="queue") as tc:
        with (
            tc.tile_pool(name="persist", bufs=1) as pp,
            tc.tile_pool(name="work", bufs=6) as wp,
        ):
            # ---- persistent SBUF tensors ----
            xt = pp.tile([P, 3, N], bf16, tag="xt")  # x^T chunks (c rows)
            wqkv = pp.tile([P, 3, 3 * C], bf16, tag="wqkv")
            wproj = pp.tile([P, 3, C], bf16, tag="wproj")
            qk = pp.tile([P, 6, N], bf16, tag="qk")  # Q^T (0..2), K^T (3..5)
            # half-swapped mirror of qk: qm[0:64,c]=qk[64:128,c] and vice
            # versa, so both PE row-halves can stream any head (dual-row
            # tiled score matmuls run two key chunks concurrently)
            qm = pp.tile([P, 6, N], bf16, tag="qm")
            vaug = pp.tile([P, NCH, H, D + 1], bf16, tag="vaug")
            ot = pp.tile([P, 3, N], bf16, tag="ot")  # normalized attn out ^T
            bias = pp.tile([P, NCH], f32, tag="bias")
            dfix = pp.tile([P, NCH, P], bf16, tag="dfix")
            ident = pp.tile([P, P], bf16, tag="ident")
            bvec = pp.tile([1, C], bf16, tag="bvec")
            ones = pp.tile([1, P], bf16, tag="ones")
            negb = pp.tile([P, 1], f32, tag="negb")  # band exp bias (-30)

            # warm-up constants first: the PE dummy matmuls and the ACT
            # table load must not wait behind the input DMA queue
            nc.vector.memset(ones[:, :], 1.0)
            nc.vector.memset(negb[:, :], NEG)
            warm = pp.tile([1, 1], f32, tag="warm")
            nc.scalar.activation(warm[:, :], negb[0:1, :], AF.Exp)
            wrow = pp.tile([1, 512], bf16, tag="wrow")
            nc.vector.memset(wrow[:, :], 0.0)

            # DMA order follows the upfront block's dependency chain: the
            # first qkv unit needs xt[:, :, 0:512] + w_qkv cc0 columns.
            xr = xT_d[:].rearrange("(a p) n -> p a n", p=P)
            qr_ = wqkvT_d[:].rearrange("(a p) n -> p a n", p=P)
            pr = wprojT_d[:].rearrange("(a p) n -> p a n", p=P)
            for c in range(3):
                nc.sync.dma_start(xt[:, c, :512], xr[:, c, :512])
            for o, w in ((0, P), (3 * P, P)):  # w_qkv cc0, cc3 columns
                for c in range(3):
                    nc.sync.dma_start(
                        wqkv[:, c, o : o + w], qr_[:, c, o : o + w]
                    )
            for c in range(3):
                nc.sync.dma_start(xt[:, c, 512:1024], xr[:, c, 512:1024])
            nc.sync.dma_start(bias[:, :], bias_d[:])
            nc.sync.dma_start(ident[:, :], ident_d[:])
            for c in range(3):  # V columns
                nc.sync.dma_start(
                    wqkv[:, c, 2 * C : 3 * C], qr_[:, c, 2 * C : 3 * C]
                )
            for c in range(3):
                nc.sync.dma_start(xt[:, c, 1024:], xr[:, c, 1024:])
            db = min(8, NCH)
            nc.sync.dma_start(dfix[:, :db, :], dfix_d[:][:, :db, :])
            for o, w in ((P, 2 * P), (4 * P, 2 * P)):  # cc1/2, cc4/5 cols
                for c in range(3):
                    nc.sync.dma_start(
                        wqkv[:, c, o : o + w], qr_[:, c, o : o + w]
                    )
            nc.sync.dma_start(dfix[:, db:, :], dfix_d[:][:, db:, :])
            for c in range(3):
                nc.sync.dma_start(wproj[:, c, :], pr[:, c, :])
            nc.sync.dma_start(bvec[:, :], bvec_d[:])

            # ---- qkv / V / mirror unit helpers ----
            # All projection work is emitted as PE filler units inside the
            # attention pass stream (PSUM tag "sc" rotation), so the exp
            # pipeline starts as soon as the first operands land.
            def _mirror(c, lo, hi):
                nc.gpsimd.dma_start(qm[64:128, c, lo:hi], qk[0:64, c, lo:hi])
                nc.gpsimd.dma_start(qm[0:64, c, lo:hi], qk[64:128, c, lo:hi])

            # ---- phase 2: attention ----
            # Piece-scoped head passes: for each (head, query-piece) the KC
            # kept chunks stream scores->exp->vmm with sc triple-buffered
            # (6 banks) while ONE out accumulator (2 banks) persists.  The
            # accumulator is released by a fast DVE copy to SBUF staging;
            # the normalization chain runs off the critical path.
            with (
                tc.tile_pool(name="outps", bufs=1, space="PSUM") as ops,
                tc.tile_pool(name="scps", bufs=3, space="PSUM") as sps,
            ):
                # last-writer bookkeeping per (piece, sub-region)
                last_band = {}
                kept_is_last = {}
                for pi, (qo, qw) in enumerate(QPIECES):
                    for (so, sw) in _subsplit(qo, qw):
                        js = [j for j in range(BSTART, NCH)
                              if so <= j * P and j * P + CHS[j] <= so + sw]
                        last_band[(pi, so)] = max(js) if js else None
                        kept_is_last[(pi, so)] = not js

                passes = [(h, pi) for pi in range(NPIECE) for h in range(H)]
                scmap = {}

                def _qkv_unit(cc, qo, qw):
                    def emit():
                        ps = sps.tile([P, 512], f32, tag="sc",
                                      name=f"qp{cc}_{qo}")
                        for c in range(3):
                            nc.tensor.matmul(
                                ps[:, :qw],
                                wqkv[:, c, cc * P : (cc + 1) * P],
                                xt[:, c, qo : qo + qw],
                                start=(c == 0),
                                stop=(c == 2),
                            )
                        nc.vector.tensor_copy(
                            qk[:, cc, qo : qo + qw], ps[:, :qw]
                        )
                    return emit

                def _v_unit(i):
                    def emit():
                        m = CHS[i]
                        ps = sps.tile([P, C], f32, tag="sc", name=f"v{i}")
                        for c in range(3):
                            nc.tensor.matmul(
                                ps[:m, :],
                                xt[:, c, i * P : i * P + m],
                                wqkv[:, c, 2 * C : 3 * C],
                                start=(c == 0),
                                stop=(c == 2),
                            )
                        nc.vector.tensor_copy(
                            vaug[:m, i, :, 0:D],
                            ps[:m, :].rearrange("p (h d) -> p h d", h=H),
                        )
                        nc.vector.memset(vaug[:m, i, :, D : D + 1], 1.0)
                    return emit

                def emit_scores_g(h, pi, i, half=None):
                    # half 0 -> PE rows 0-63 (tile T0), half 1 -> rows
                    # 64-127 (T8); the head's natural half uses qk, the
                    # other uses the qm mirror.  Adjacent T0/T8 matmuls
                    # execute concurrently (independent row groups).
                    kc, qc = 3 + h // 2, h // 2
                    if half is None:
                        half = h % 2
                    src_t = qk if half == (h % 2) else qm
                    r = half * D
                    qo, qw = QPIECES[pi]
                    m = CHS[i]
                    sc = sps.tile([P, SLOTW], f32, tag="sc",
                                  name=f"sc{h}{pi}_{i}")
                    subs = _subsplit(qo, qw)
                    for si, (so, sw) in enumerate(subs):
                        nc.tensor.matmul(
                            sc[:m, so - qo : so - qo + sw],
                            src_t[r : r + D, kc, i * P : i * P + m],
                            src_t[r : r + D, qc, so : so + sw],
                            start=True,
                            stop=si == len(subs) - 1,
                        )
                    scmap[(h, pi, i)] = sc

                # deferred fillers keyed by (pass_idx, chunk_i): remaining
                # qkv channel units, V chunks, and later-wave channels
                fillers = {}

                def _add(key, fn):
                    fillers.setdefault(key, []).append(fn)

                # Passes 0-1 (h0/h1, piece 0) run during the input DMA
                # stream with natural-half single scores (no mirror dep);
                # remaining qkv channels, V chunks and mirrors are fillers
                # ordered to match DMA arrival.  Q channels mirror both
                # column ranges; K channels only [0,1024) (kept keys).
                KCm = KC - 1

                def _ki(i):
                    return min(i, KCm)

                _add((0, _ki(0)), _v_unit(1))
                _add((0, _ki(0)), _v_unit(2))
                _add((0, _ki(1)), _v_unit(3))
                _add((0, _ki(1)), _qkv_unit(3, 512, 512))
                _add((0, _ki(2)), _v_unit(4))
                _add((0, _ki(2)), _v_unit(5))
                _add((0, _ki(3)), _v_unit(6))
                _add((0, _ki(3)), _v_unit(7))
                _add((0, _ki(4)), _qkv_unit(0, 1024, 512))
                _add((0, _ki(4)), _qkv_unit(3, 1024, 512))
                _add((0, _ki(5)), _v_unit(8))
                _add((0, _ki(5)), _v_unit(9))
                _add((0, _ki(5)), _qkv_unit(0, 1536, N - 1536))
                _add((0, _ki(6)), _v_unit(10))
                _add((0, _ki(6)), _v_unit(11))
                _add((0, _ki(6)), _qkv_unit(3, 1536, N - 1536))
                _add((1, _ki(0)), _v_unit(12))
                _add((1, _ki(0)), lambda: _mirror(3, 0, 1024))
                _add((1, _ki(0)), lambda: _mirror(0, 0, 1024))
                _add((1, _ki(0)), lambda: _mirror(0, 1024, N))
                for wave, (ccq, cck) in enumerate(((1, 4), (2, 5))):
                    units = [
                        _qkv_unit(ccq, 0, 512),
                        _qkv_unit(ccq, 512, 512),
                        lambda c=ccq: _mirror(c, 0, 1024),
                        _qkv_unit(cck, 0, 512),
                        _qkv_unit(cck, 512, 512),
                        lambda c=cck: _mirror(c, 0, 1024),
                        _qkv_unit(ccq, 1024, 512),
                        _qkv_unit(ccq, 1536, N - 1536),
                        lambda c=ccq: _mirror(c, 1024, N),
                        _qkv_unit(cck, 1024, 512),
                        _qkv_unit(cck, 1536, N - 1536),
                    ]
                    slots = [(1 + wave, i) for i in range(1, KC)]
                    step = max(1, len(slots) // len(units))
                    for u, unit in enumerate(units):
                        key = slots[min(u * step, len(slots) - 1)]
                        _add(key, unit)

                # piece-0 projections become PE fillers spread across the
                # piece-1 passes (ot piece 0 is complete after pass H-1)
                p0_chunks = [j for j in range(NCH)
                             if j * P + CHS[j] <= QPIECES[0][1]]
                for k, j in enumerate(p0_chunks):
                    key = (H + min(k, H - 1), 2)
                    fillers.setdefault(key, []).append(
                        lambda j=j: emit_proj(j, use_dve=(j % 2 == 0)))

                def emit_pass(idx):
                    h, pi = passes[idx]
                    kc, kr = 3 + h // 2, (h % 2) * D
                    qc, qr = h // 2, (h % 2) * D
                    qo, qw = QPIECES[pi]
                    subs = _subsplit(qo, qw)
                    outs = ops.tile([D + 1, SLOTW], f32, tag="outT",
                                    name=f"o{h}_{pi}")
                    ets = {}
                    # chunk-pair schedule: after exp(i) emit scores for the
                    # next pair (dual-row tiled, both halves) or single.
                    # Passes 0-1 run before the qm mirrors exist -> singles.
                    sched = {}
                    if idx < 2:
                        for a in range(1, KC):
                            sched[a - 1] = (a,)
                    else:
                        a = 1
                        while a < KC:
                            if a + 1 < KC:
                                sched[a - 1] = (a, a + 1)
                                a += 2
                            else:
                                sched[a - 1] = (a,)
                                a += 1

                    for i in range(KC):
                        m = CHS[i]
                        et = wp.tile([P, SLOTW], bf16, tag="et",
                                     name=f"et{h}{pi}_{i}")
                        nc.scalar.activation(
                            et[:m, :qw],
                            scmap.pop((h, pi, i))[:m, :qw],
                            AF.Exp,
                            bias=bias[:m, i : i + 1],
                            scale=SCALE,
                        )
                        ets[i] = et
                        for k, nx in enumerate(sched.get(i, ())):
                            emit_scores_g(h, pi, nx,
                                          half=(k if len(sched[i]) == 2
                                                else None))
                        for (so, sw) in subs:
                            nc.tensor.matmul(
                                outs[:, so - qo : so - qo + sw],
                                vaug[:m, i, h, :],
                                ets[i][:m, so - qo : so - qo + sw],
                                start=(i == 0),
                                stop=(i == KC - 1 and kept_is_last[(pi, so)]),
                            )
                        del ets[i]
                        for unit in fillers.get((idx, i), []):
                            unit()

                    # band (diag-only) blocks of this piece: dfix written
                    # first per 512-bank (start=True sets has_written for
                    # the whole region), then the diag score matmuls
                    # accumulate, then ONE wide exp per <=8 blocks.
                    bj = [j for j in range(BSTART, NCH)
                          if qo <= j * P and j * P + CHS[j] <= qo + qw]
                    bscr = []
                    for g0 in range(0, len(bj), 8):
                        grp8 = bj[g0 : g0 + 8]
                        scb = sps.tile([P, SLOTW], f32, tag="sc",
                                       name=f"sb{h}{pi}_{g0}")
                        for b0 in range(0, len(grp8), 4):
                            grp = grp8[b0 : b0 + 4]
                            off = b0 * P
                            nc.tensor.matmul(
                                scb[:, off : off + len(grp) * P],
                                ident[:, :],
                                dfix[:, grp[0] : grp[0] + len(grp), :],
                                start=True,
                                stop=False,
                            )
                            # natural half only: concurrent row tiles must
                            # not write the same PSUM bank
                            r = (h % 2) * D
                            for bi, j in enumerate(grp):
                                m = CHS[j]
                                nc.tensor.matmul(
                                    scb[:m, off + bi * P : off + bi * P + m],
                                    qk[r : r + D, kc, j * P : j * P + m],
                                    qk[r : r + D, qc, j * P : j * P + m],
                                    start=False,
                                    stop=(bi == len(grp) - 1),
                                )
                        bscr.append((grp8, scb))

                    # pre-emit the next pass's first scores so the PE can
                    # run them while this pass's band exp drains
                    if idx + 1 < len(passes):
                        nh, npi = passes[idx + 1]
                        emit_scores_g(nh, npi, 0)

                    for grp8, scb in bscr:
                        etb = wp.tile([P, SLOTW], bf16, tag="et",
                                      name=f"eb{h}{pi}_{grp8[0]}")
                        wtot = len(grp8) * P
                        nc.scalar.activation(
                            etb[:, :wtot],
                            scb[:, :wtot],
                            AF.Exp,
                            bias=negb[:, :],
                            scale=SCALE,
                        )
                        for bi, j in enumerate(grp8):
                            m = CHS[j]
                            for (so, sw) in subs:
                                if so <= j * P and j * P + CHS[j] <= so + sw:
                                    nc.tensor.matmul(
                                        outs[:, j * P - qo : j * P - qo + m],
                                        vaug[:m, j, h, :],
                                        etb[:m, bi * P : bi * P + m],
                                        start=False,
                                        stop=last_band[(pi, so)] == j,
                                    )

                    # fast PSUM release: one DVE copy to SBUF staging frees
                    # the accumulator; the normalization chain (reciprocal,
                    # gpsimd broadcast, multiply) runs off the critical path
                    stg = wp.tile([D + 1, SLOTW], f32, tag="stg", bufs=3,
                                  name=f"st{h}{pi}")
                    nc.vector.tensor_copy(stg[:, :qw], outs[:, :qw])
                    srow = wp.tile([1, SLOTW], f32, tag="srow", bufs=3,
                                   name=f"sr{h}{pi}")
                    nc.vector.tensor_copy(srow[:, :qw], stg[D : D + 1, :qw])
                    rinv = wp.tile([1, SLOTW], f32, tag="rinv", bufs=3,
                                   name=f"ri{h}{pi}")
                    nc.vector.reciprocal_approx_fast(
                        rinv[:, :qw], srow[:, :qw]
                    )
                    rbr = wp.tile([D, SLOTW], f32, tag="rbr", bufs=3,
                                  name=f"rr{h}{pi}")
                    nc.gpsimd.partition_broadcast(rbr[:, :qw], rinv[:, :qw])
                    nc.vector.tensor_mul(
                        ot[qr : qr + D, qc, qo : qo + qw],
                        stg[0:D, :qw],
                        rbr[:, :qw],
                    )

                    if h == H - 1 and pi == NPIECE - 1:
                        for j in range(NCH):
                            if qo <= j * P and j * P + CHS[j] <= qo + qw:
                                emit_proj(j, use_dve=(j % 2 == 0))

                def emit_proj(j, use_dve):
                    m = CHS[j]
                    yp = sps.tile([P, SLOTW], f32, tag="sc", name=f"yp{j}")
                    for c in range(3):
                        nc.tensor.matmul(
                            yp[:m, :C],
                            ot[:, c, j * P : j * P + m],
                            wproj[:, c, :],
                            start=(c == 0),
                            stop=(c == 2 and not HASB),
                        )
                    if HASB:
                        nc.tensor.matmul(
                            yp[:m, :C],
                            ones[:, :m],
                            bvec[:, :],
                            start=False,
                            stop=True,
                        )
                    ys = wp.tile([P, C], f32, tag="ys", name=f"ys{j}")
                    if use_dve:
                        nc.vector.tensor_copy(ys[:m, :], yp[:m, :C])
                    else:
                        nc.scalar.copy(ys[:m, :], yp[:m, :C])
                    # split per chunk across two hw queues (per-queue bw is
                    # ~23 GB/s; one 196KB chunk would be an ~8us drain) and
                    # dispatch from compute engines to spare the sync queue
                    hm = (m + 1) // 2
                    nc.gpsimd.dma_start(
                        out_d[j * P : j * P + hm, :], ys[0:hm, :]
                    )
                    nc.sync.dma_start(
                        out_d[j * P + hm : j * P + m, :], ys[hm:m, :]
                    )

                # prepass: PE warm-up dummies while the first DMAs land,
                # then the minimal qkv units for (h0, piece0) chunk 0
                dz = sps.tile([P, 512], f32, tag="sc", name="warmmm")
                for _ in range(8):
                    nc.tensor.matmul(
                        dz[:, :], ones[:, :], wrow[:, :],
                        start=True, stop=True,
                    )
                junk = pp.tile([P, 1], f32, tag="junk")
                nc.vector.tensor_copy(junk[:, :], dz[:, 0:1])
                _qkv_unit(0, 0, 512)()
                _qkv_unit(3, 0, 512)()
                _qkv_unit(0, 512, 512)()
                emit_scores_g(*passes[0], 0)
                _v_unit(0)()
                for idx in range(len(passes)):
                    emit_pass(idx)

    nc.finalize()
    return nc


def _prep_core_inputs(x_b, p_b, wqkvT, wprojT, bvec, ident):
    """Permute tokens kept-keys-first; build exp-bias and diag-fix tensors.
    Returns (in_map, perm)."""
    import ml_dtypes

    bf16 = ml_dtypes.bfloat16
    perm = np.argsort(-p_b, kind="stable")
    xT = np.ascontiguousarray(x_b[perm].T).astype(bf16)
    p_perm = p_b[perm].astype(np.float32)
    pad = NCH * P - N
    p_pad = np.concatenate([p_perm, np.zeros(pad, np.float32)])
    # bias_exp[r, i] = -30 * (1 - p[i*128 + r]) per key chunk
    bias = (NEG * (1.0 - p_pad)).reshape(NCH, P).T.copy()
    # dfix[:, i, :] = diag(240 * (1 - p_chunk_i)) as bf16
    dfix = np.zeros((P, NCH, P), np.float32)
    for i in range(NCH):
        chunk = p_pad[i * P : (i + 1) * P]
        np.fill_diagonal(dfix[:, i, :], DIAGV * (1.0 - chunk))
    return {
        "xT": xT,
        "wqkvT": wqkvT,
        "wprojT": wprojT,
        "bias_exp": np.ascontiguousarray(bias),
        "dfix": dfix.astype(bf16),
        "ident": ident,
        "bvec": bvec,
    }, perm


def _install_ntff_hook():
    """The container's antenv package lacks axon_hooks; recreate the NTFF
    profile hook (mirrors trn_agent_boot) so trace=True yields exec_time."""
    import types
    import ctypes
    import contextlib

    if "antenv.axon_hooks" in sys.modules:
        return
    so_path = "/opt/axon/libaxon_pjrt.so"
    mod = types.ModuleType("antenv.axon_hooks")
    state = {"hook": None}
    mod.set_axon_ntff_profile_hook = lambda h: state.__setitem__("hook", h)
    mod.get_axon_ntff_profile_hook = lambda: state["hook"]
    sys.modules["antenv.axon_hooks"] = mod

    try:
        lib = ctypes.CDLL(so_path)
    except OSError:
        return
    if not hasattr(lib, "axon_start_nrt_profile"):
        return
    lib.axon_start_nrt_profile.argtypes = [
        ctypes.POINTER(ctypes.c_int64),
        ctypes.c_size_t,
    ]
    lib.axon_start_nrt_profile.restype = ctypes.c_int64
    lib.axon_stop_nrt_profile.argtypes = [ctypes.c_char_p]
    lib.axon_stop_nrt_profile.restype = ctypes.c_int64

    @contextlib.contextmanager
    def _hook(output_dir, device_ids):
        import jax

        jax.devices()
        if device_ids:
            ids = (ctypes.c_int64 * len(device_ids))(*device_ids)
            rc = lib.axon_start_nrt_profile(ids, len(device_ids))
        else:
            rc = lib.axon_start_nrt_profile(None, 0)
        if rc != 0:
            raise RuntimeError(f"axon_start_nrt_profile rc={rc}")
        try:
            yield
        finally:
            n = lib.axon_stop_nrt_profile(str(output_dir).encode())
            print(f"profile: {n} file(s) written to {output_dir}", file=sys.stderr)

    state["hook"] = _hook


def kernel(x, vis_tube, w_qkv, w_proj, b_proj, _trace=False):
    from concourse.bass_utils import run_bass_kernel_spmd

    import ml_dtypes

    if _trace:
        _install_ntff_hook()

    bf16 = ml_dtypes.bfloat16
    x = np.asarray(x, np.float32)
    p = np.asarray(vis_tube, np.float32)[:, :, 0]
    keeps = (p > 0.5).sum(axis=1)  # kept keys per batch
    KC = max(1, int(-(-keeps.max() // P)))  # chunks containing kept keys
    BSTART = int(keeps.min() // P)  # first chunk containing a dropped key

    HASB = bool(np.any(np.asarray(b_proj)))
    key = (KC, BSTART, HASB)
    if _CACHE.get("key") != key:
        _CACHE["nc"] = _build_nc(KC, BSTART, HASB)
        _CACHE["key"] = key
    nc = _CACHE["nc"]

    wqkvT = np.ascontiguousarray(np.asarray(w_qkv).T).astype(bf16)
    wprojT = np.ascontiguousarray(np.asarray(w_proj).T).astype(bf16)
    bvec = np.asarray(b_proj).reshape(1, C).astype(np.float32).astype(bf16)
    ident = np.eye(P, dtype=np.float32).astype(bf16)
    in_maps, perms = [], []
    for b in range(B):
        im, perm = _prep_core_inputs(x[b], p[b], wqkvT, wprojT, bvec, ident)
        in_maps.append(im)
        perms.append(perm)
    res = run_bass_kernel_spmd(nc, in_maps, core_ids=list(range(B)), trace=_trace)
    out = np.empty((B, N, C), np.float32)
    for b in range(B):
        out[b][perms[b]] = np.asarray(res.results[b]["out"], np.float32)
    if _trace:
        _CACHE["last_result"] = res
    return out


# revision 30
# speedup vs baseline: 1.0124x; 1.0008x over previous
"""Trainium2 Bass kernel for policy-masked attention (nn_Attention_5007931867377).

Reference computation (per batch b):
    qkv = x @ w_qkv.T ; split into q,k,v heads [H=6, N=1568, D=64]
    s   = (q @ k.T) * D**-0.5
    mask[m] visibility per key + diagonal always kept
    e   = exp(s - max) * mask ; attn = (e + EPS/N)/(sum e + EPS)
    out = (attn @ v) concat heads @ w_proj.T + b_proj

Strategy: pure data parallel, one batch element per NeuronCore (8 cores).
Per-core dataflow (everything transposed so softmax reductions are on the
free axis and e^T feeds the V-matmul without on-chip transposes):
  - host passes x^T (bf16); on chip: Q^T,K^T in [c_out, n] layout, V in [n, d]
  - scores^T[m, q] = K^T.T @ Q^T  (PSUM, per 128-key chunk x query piece)
  - key-mask folded into exp as per-partition bias (-30 * (1-p));
    diagonal-keep folded into scores via +240*(1-p) diag matmul (exp scale
    0.125 makes that +30, cancelling the -30 bias exactly on the diagonal)
  - e^T = exp(0.125*s + bias) ACT -> SBUF (bf16)
  - V augmented with a ones column: outT_aug[65, q] = V_aug.T @ e^T gives
    attention output rows 0..63 and the softmax denominator in row 64
  - normalize with reciprocal_approx_fast + gpsimd partition_broadcast
  - proj: y[n, :] = oT.T @ w_proj.T (+ bias via K=1 ones matmul), DMA out.

Schedule: piece-scoped head passes (h, query-piece) over the KC kept key
chunks, software-pipelined so PE and ACT overlap:
  - exp is issued WIDE (768/800-col pieces, each 2 PSUM banks) to amortize
    the ~200-cycle ACTIVATE init; per-head band (diag-only) blocks batch
    into one [128, NB*128] tile with a single exp (dfix written first per
    bank so the diag score matmuls accumulate onto it).
  - scores are dual-row-tiled: a half-swapped mirror (qm) of Q^T/K^T lets
    chunk pairs run concurrently on PE row halves T0/T8.
  - PSUM: 1 out accumulator (2 banks, bufs=1) + sc pool (2 banks x bufs=3);
    the accumulator is released by one DVE copy to SBUF staging and the
    norm chain (recip + gpsimd broadcast + mul) runs off the critical path.
  - qkv cc1/4 + cc2/5 units and piece-0 projections interleave as PE
    fillers inside later passes; out DMA splits across two queue groups.
All matmuls bf16 (HAM-warm PE).  Max-subtraction is dropped: scores ~
N(0,1), |s| < ~6.
"""

import sys

if "/opt/trn_rl_repo" not in sys.path:
    sys.path.insert(0, "/opt/trn_rl_repo")

import numpy as np

B, N, C, H = 8, 1568, 384, 6
D = C // H  # 64
SCALE = D ** -0.5  # 0.125
EPS = 1e-6
NEG = -30.0  # masked-key exp bias (exp(-30) ~ 9e-14, way below EPS/N)
DIAGV = -NEG / SCALE  # 240.0 added to diagonal scores, cancels bias exactly

P = 128
NCH = (N + P - 1) // P  # 13 key/token chunks (12 x 128 + 1 x 32)
CHS = [min(P, N - i * P) for i in range(NCH)]
# query pieces: chunk-aligned (multiples of 128 so band blocks nest), each
# <= 2 PSUM banks; matmuls subsplit at 512 (bank bounds) inside a piece
QPIECES = [(0, 768), (768, 800)]
SLOTW = 1024  # PSUM slot width (2 banks f32)
NPIECE = len(QPIECES)


def _subsplit(qo, qw, step=512):
    return [(qo + o, min(step, qw - o)) for o in range(0, qw, step)]


_CACHE = {}


def _build_nc(KC, BSTART, HASB):
    """Build the SPMD program.

    Tokens are permuted host-side so policy-kept keys come first. KC = number
    of 128-key chunks holding any kept key (full scores+softmax+V path);
    chunks BSTART..NCH-1 contain dropped keys, which are visible only to
    their own query (diagonal) — handled by small diagonal-block tasks
    (constant -30 exp bias + dfix diag matmul restores exactly the
    (1-p) diagonal entries), batched per head into one wide exp.
    """
    import concourse.tile as tile
    from concourse import bacc, mybir

    dt = mybir.dt
    f32 = dt.float32
    bf16 = dt.bfloat16
    AF = mybir.ActivationFunctionType

    NB = NCH - BSTART  # number of band (diag-only) chunks

    nc = bacc.Bacc()

    xT_d = nc.declare_dram_parameter("xT", [C, N], bf16, isOutput=False)
    wqkvT_d = nc.declare_dram_parameter("wqkvT", [C, 3 * C], bf16, isOutput=False)
    wprojT_d = nc.declare_dram_parameter("wprojT", [C, C], bf16, isOutput=False)
    bias_d = nc.declare_dram_parameter("bias_exp", [P, NCH], f32, isOutput=False)
    dfix_d = nc.declare_dram_parameter("dfix", [P, NCH, P], bf16, isOutput=False)
    ident_d = nc.declare_dram_parameter("ident", [P, P], bf16, isOutput=False)
    bvec_d = nc.declare_dram_parameter("bvec", [1, C], bf16, isOutput=False)
    out_d = nc.declare_dram_parameter("out", [N, C], f32, isOutput=True)

    with tile.TileContext(nc, pool_alloc_mode="queue") as tc:
        with (
            tc.tile_pool(name="persist", bufs=1) as pp,
            tc.tile_pool(name="work", bufs=6) as wp,
        ):
            # ---- persistent SBUF tensors ----
            xt = pp.tile([P, 3, N], bf16, tag="xt")  # x^T chunks (c rows)
            wqkv = pp.tile([P, 3, 3 * C], bf16, tag="wqkv")
            wproj = pp.tile([P, 3, C], bf16, tag="wproj")
            qk = pp.tile([P, 6, N], bf16, tag="qk")  # Q^T (0..2), K^T (3..5)
            # half-swapped mirror of qk: qm[0:64,c]=qk[64:128,c] and vice
            # versa, so both PE row-halves can stream any head (dual-row
            # tiled score matmuls run two key chunks concurrently)
            qm = pp.tile([P, 6, N], bf16, tag="qm")
            vaug = pp.tile([P, NCH, H, D + 1], bf16, tag="vaug")
            ot = pp.tile([P, 3, N], bf16, tag="ot")  # normalized attn out ^T
            bias = pp.tile([P, NCH], f32, tag="bias")
            dfix = pp.tile([P, NCH, P], bf16, tag="dfix")
            ident = pp.tile([P, P], bf16, tag="ident")
            bvec = pp.tile([1, C], bf16, tag="bvec")
            ones = pp.tile([1, P], bf16, tag="ones")
            negb = pp.tile([P, 1], f32, tag="negb")  # band exp bias (-30)

            # warm-up constants first: the PE dummy matmuls and the ACT
            # table load must not wait behind the input DMA queue
            nc.vector.memset(ones[:, :], 1.0)
            nc.vector.memset(negb[:, :], NEG)
            warm = pp.tile([1, 1], f32, tag="warm")
            nc.scalar.activation(warm[:, :], negb[0:1, :], AF.Exp)
            wrow = pp.tile([1, 512], bf16, tag="wrow")
            nc.vector.memset(wrow[:, :], 0.0)

            # DMA order follows the upfront block's dependency chain: the
            # first qkv unit needs xt[:, :, 0:512] + w_qkv cc0 columns.
            xr = xT_d[:].rearrange("(a p) n -> p a n", p=P)
            qr_ = wqkvT_d[:].rearrange("(a p) n -> p a n", p=P)
            pr = wprojT_d[:].rearrange("(a p) n -> p a n", p=P)
            for c in range(3):
                nc.sync.dma_start(xt[:, c, :512], xr[:, c, :512])
            for o, w in ((0, P), (3 * P, P)):  # w_qkv cc0, cc3 columns
                for c in range(3):
                    nc.sync.dma_start(
                        wqkv[:, c, o : o + w], qr_[:, c, o : o + w]
                    )
            for c in range(3):
                nc.sync.dma_start(xt[:, c, 512:1024], xr[:, c, 512:1024])
            nc.sync.dma_start(bias[:, :], bias_d[:])
            nc.sync.dma_start(ident[:, :], ident_d[:])
            for c in range(3):  # V columns
                nc.sync.dma_start(
                    wqkv[:, c, 2 * C : 3 * C], qr_[:, c, 2 * C : 3 * C]
                )
            for c in range(3):
                nc.sync.dma_start(xt[:, c, 1024:], xr[:, c, 1024:])
            db = min(8, NCH)
            nc.sync.dma_start(dfix[:, :db, :], dfix_d[:][:, :db, :])
            for o, w in ((P, 2 * P), (4 * P, 2 * P)):  # cc1/2, cc4/5 cols
                for c in range(3):
                    nc.sync.dma_start(
                        wqkv[:, c, o : o + w], qr_[:, c, o : o + w]
                    )
            nc.sync.dma_start(dfix[:, db:, :], dfix_d[:][:, db:, :])
            for c in range(3):
                nc.sync.dma_start(wproj[:, c, :], pr[:, c, :])
            nc.sync.dma_start(bvec[:, :], bvec_d[:])

            # ---- qkv / V / mirror unit helpers ----
            # All projection work is emitted as PE filler units inside the
            # attention pass stream (PSUM tag "sc" rotation), so the exp
            # pipeline starts as soon as the first operands land.
            def _mirror(c, lo, hi):
                nc.gpsimd.dma_start(qm[64:128, c, lo:hi], qk[0:64, c, lo:hi])
                nc.gpsimd.dma_start(qm[0:64, c, lo:hi], qk[64:128, c, lo:hi])

            # ---- phase 2: attention ----
            # Piece-scoped head passes: for each (head, query-piece) the KC
            # kept chunks stream scores->exp->vmm with sc triple-buffered
            # (6 banks) while ONE out accumulator (2 banks) persists.  The
            # accumulator is released by a fast DVE copy to SBUF staging;
            # the normalization chain runs off the critical path.
            with (
                tc.tile_pool(name="outps", bufs=1, space="PSUM") as ops,
                tc.tile_pool(name="scps", bufs=3, space="PSUM") as sps,
            ):
                # last-writer bookkeeping per (piece, sub-region)
                last_band = {}
                kept_is_last = {}
                for pi, (qo, qw) in enumerate(QPIECES):
                    for (so, sw) in _subsplit(qo, qw):
                        js = [j for j in range(BSTART, NCH)
                              if so <= j * P and j * P + CHS[j] <= so + sw]
                        last_band[(pi, so)] = max(js) if js else None
                        kept_is_last[(pi, so)] = not js

                passes = [(h, pi) for pi in range(NPIECE) for h in range(H)]
                scmap = {}

                def _qkv_unit(cc, qo, qw):
                    def emit():
                        ps = sps.tile([P, 512], f32, tag="sc",
                                      name=f"qp{cc}_{qo}")
                        for c in range(3):
                            nc.tensor.matmul(
                                ps[:, :qw],
                                wqkv[:, c, cc * P : (cc + 1) * P],
                                xt[:, c, qo : qo + qw],
                                start=(c == 0),
                                stop=(c == 2),
                            )
                        nc.vector.tensor_copy(
                            qk[:, cc, qo : qo + qw], ps[:, :qw]
                        )
                    return emit

                def _v_unit(i):
                    def emit():
                        m = CHS[i]
                        ps = sps.tile([P, C], f32, tag="sc", name=f"v{i}")
                        for c in range(3):
                            nc.tensor.matmul(
                                ps[:m, :],
                                xt[:, c, i * P : i * P + m],
                                wqkv[:, c, 2 * C : 3 * C],
                                start=(c == 0),
                                stop=(c == 2),
                            )
                        nc.vector.tensor_copy(
                            vaug[:m, i, :, 0:D],
                            ps[:m, :].rearrange("p (h d) -> p h d", h=H),
                        )
                        nc.vector.memset(vaug[:m, i, :, D : D + 1], 1.0)
                    return emit

                def emit_scores_g(h, pi, i, half=None):
                    # half 0 -> PE rows 0-63 (tile T0), half 1 -> rows
                    # 64-127 (T8); the head's natural half uses qk, the
                    # other uses the qm mirror.  Adjacent T0/T8 matmuls
                    # execute concurrently (independent row groups).
                    kc, qc = 3 + h // 2, h // 2
                    if half is None:
                        half = h % 2
                    src_t = qk if half == (h % 2) else qm
                    r = half * D
                    qo, qw = QPIECES[pi]
                    m = CHS[i]
                    sc = sps.tile([P, SLOTW], f32, tag="sc",
                                  name=f"sc{h}{pi}_{i}")
                    subs = _subsplit(qo, qw)
                    for si, (so, sw) in enumerate(subs):
                        nc.tensor.matmul(
                            sc[:m, so - qo : so - qo + sw],
                            src_t[r : r + D, kc, i * P : i * P + m],
                            src_t[r : r + D, qc, so : so + sw],
                            start=True,
                            stop=si == len(subs) - 1,
                        )
                    scmap[(h, pi, i)] = sc

                # deferred fillers keyed by (pass_idx, chunk_i): remaining
                # qkv channel units, V chunks, and later-wave channels
                fillers = {}

                def _add(key, fn):
                    fillers.setdefault(key, []).append(fn)

                # Passes 0-1 (h0/h1, piece 0) run during the input DMA
                # stream with natural-half single scores (no mirror dep);
                # remaining qkv channels, V chunks and mirrors are fillers
                # ordered to match DMA arrival.  Q channels mirror both
                # column ranges; K channels only [0,1024) (kept keys).
                KCm = KC - 1

                def _ki(i):
                    return min(i, KCm)

                _add((0, _ki(0)), _v_unit(1))
                _add((0, _ki(0)), _v_unit(2))
                _add((0, _ki(1)), _v_unit(3))
                _add((0, _ki(1)), _qkv_unit(3, 512, 512))
                _add((0, _ki(2)), _v_unit(4))
                _add((0, _ki(2)), _v_unit(5))
                _add((0, _ki(3)), _v_unit(6))
                _add((0, _ki(3)), _v_unit(7))
                _add((0, _ki(4)), _qkv_unit(0, 1024, 512))
                _add((0, _ki(4)), _qkv_unit(3, 1024, 512))
                _add((0, _ki(5)), _v_unit(8))
                _add((0, _ki(5)), _v_unit(9))
                _add((0, _ki(5)), _qkv_unit(0, 1536, N - 1536))
                _add((0, _ki(6)), _v_unit(10))
                _add((0, _ki(6)), _v_unit(11))
                _add((0, _ki(6)), _qkv_unit(3, 1536, N - 1536))
                _add((1, _ki(0)), _v_unit(12))
                _add((1, _ki(0)), lambda: _mirror(3, 0, 1024))
                _add((1, _ki(0)), lambda: _mirror(0, 0, 1024))
                _add((1, _ki(0)), lambda: _mirror(0, 1024, N))
                for wave, (ccq, cck) in enumerate(((1, 4), (2, 5))):
                    units = [
                        _qkv_unit(ccq, 0, 512),
                        _qkv_unit(ccq, 512, 512),
                        lambda c=ccq: _mirror(c, 0, 1024),
                        _qkv_unit(cck, 0, 512),
                        _qkv_unit(cck, 512, 512),
                        lambda c=cck: _mirror(c, 0, 1024),
                        _qkv_unit(ccq, 1024, 512),
                        _qkv_unit(ccq, 1536, N - 1536),
                        lambda c=ccq: _mirror(c, 1024, N),
                        _qkv_unit(cck, 1024, 512),
                        _qkv_unit(cck, 1536, N - 1536),
                    ]
                    slots = [(1 + wave, i) for i in range(1, KC)]
                    step = max(1, len(slots) // len(units))
                    for u, unit in enumerate(units):
                        key = slots[min(u * step, len(slots) - 1)]
                        _add(key, unit)

                # piece-0 projections become PE fillers spread across the
                # piece-1 passes (ot piece 0 is complete after pass H-1)
                p0_chunks = [j for j in range(NCH)
                             if j * P + CHS[j] <= QPIECES[0][1]]
                for k, j in enumerate(p0_chunks):
                    key = (H + min(k, H - 1), 2)
                    fillers.setdefault(key, []).append(
                        lambda j=j: emit_proj(j, use_dve=(j % 2 == 0)))

                def emit_pass(idx):
                    h, pi = passes[idx]
                    kc, kr = 3 + h // 2, (h % 2) * D
                    qc, qr = h // 2, (h % 2) * D
                    qo, qw = QPIECES[pi]
                    subs = _subsplit(qo, qw)
                    outs = ops.tile([D + 1, SLOTW], f32, tag="outT",
                                    name=f"o{h}_{pi}")
                    ets = {}
                    # chunk-pair schedule: after exp(i) emit scores for the
                    # next pair (dual-row tiled, both halves) or single.
                    # Passes 0-1 run before the qm mirrors exist -> singles.
                    sched = {}
                    if idx < 2:
                        for a in range(1, KC):
                            sched[a - 1] = (a,)
                    else:
                        a = 1
                        while a < KC:
                            if a + 1 < KC:
                                sched[a - 1] = (a, a + 1)
                                a += 2
                            else:
                                sched[a - 1] = (a,)
                                a += 1

                    for i in range(KC):
                        m = CHS[i]
                        et = wp.tile([P, SLOTW], bf16, tag="et",
                                     name=f"et{h}{pi}_{i}")
                        nc.scalar.activation(
                            et[:m, :qw],
                            scmap.pop((h, pi, i))[:m, :qw],
                            AF.Exp,
                            bias=bias[:m, i : i + 1],
                            scale=SCALE,
                        )
                        ets[i] = et
                        for k, nx in enumerate(sched.get(i, ())):
                            emit_scores_g(h, pi, nx,
                                          half=(k if len(sched[i]) == 2
                                                else None))
                        for (so, sw) in subs:
                            nc.tensor.matmul(
                                outs[:, so - qo : so - qo + sw],
                                vaug[:m, i, h, :],
                                ets[i][:m, so - qo : so - qo + sw],
                                start=(i == 0),
                                stop=(i == KC - 1 and kept_is_last[(pi, so)]),
                            )
                        del ets[i]
                        for unit in fillers.get((idx, i), []):
                            unit()

                    # band (diag-only) blocks of this piece: dfix written
                    # first per 512-bank (start=True sets has_written for
                    # the whole region), then the diag score matmuls
                    # accumulate, then ONE wide exp per <=8 blocks.
                    bj = [j for j in range(BSTART, NCH)
                          if qo <= j * P and j * P + CHS[j] <= qo + qw]
                    bscr = []
                    for g0 in range(0, len(bj), 8):
                        grp8 = bj[g0 : g0 + 8]
                        scb = sps.tile([P, SLOTW], f32, tag="sc",
                                       name=f"sb{h}{pi}_{g0}")
                        for b0 in range(0, len(grp8), 4):
                            grp = grp8[b0 : b0 + 4]
                            off = b0 * P
                            nc.tensor.matmul(
                                scb[:, off : off + len(grp) * P],
                                ident[:, :],
                                dfix[:, grp[0] : grp[0] + len(grp), :],
                                start=True,
                                stop=False,
                            )
                            # natural half only: concurrent row tiles must
                            # not write the same PSUM bank
                            r = (h % 2) * D
                            for bi, j in enumerate(grp):
                                m = CHS[j]
                                nc.tensor.matmul(
                                    scb[:m, off + bi * P : off + bi * P + m],
                                    qk[r : r + D, kc, j * P : j * P + m],
                                    qk[r : r + D, qc, j * P : j * P + m],
                                    start=False,
                                    stop=(bi == len(grp) - 1),
                                )
                        bscr.append((grp8, scb))

                    # pre-emit the next pass's first scores so the PE can
                    # run them while this pass's band exp drains
                    if idx + 1 < len(passes):
                        nh, npi = passes[idx + 1]
                        emit_scores_g(nh, npi, 0)

                    for grp8, scb in bscr:
                        etb = wp.tile([P, SLOTW], bf16, tag="et",
                                      name=f"eb{h}{pi}_{grp8[0]}")
                        wtot = len(grp8) * P
                        nc.scalar.activation(
                            etb[:, :wtot],
                            scb[:, :wtot],
                            AF.Exp,
                            bias=negb[:, :],
                            scale=SCALE,
                        )
                        for bi, j in enumerate(grp8):
                            m = CHS[j]
                            for (so, sw) in subs:
                                if so <= j * P and j * P + CHS[j] <= so + sw:
                                    nc.tensor.matmul(
                                        outs[:, j * P - qo : j * P - qo + m],
                                        vaug[:m, j, h, :],
                                        etb[:m, bi * P : bi * P + m],
                                        start=False,
                                        stop=last_band[(pi, so)] == j,
                                    )

                    # fast PSUM release: one DVE copy to SBUF staging frees
                    # the accumulator; the normalization chain (reciprocal,
                    # gpsimd broadcast, multiply) runs off the critical path
                    stg = wp.tile([D + 1, SLOTW], f32, tag="stg", bufs=3,
                                  name=f"st{h}{pi}")
                    nc.vector.tensor_copy(stg[:, :qw], outs[:, :qw])
                    srow = wp.tile([1, SLOTW], f32, tag="srow", bufs=3,
                                   name=f"sr{h}{pi}")
                    nc.vector.tensor_copy(srow[:, :qw], stg[D : D + 1, :qw])
                    rinv = wp.tile([1, SLOTW], f32, tag="rinv", bufs=3,
                                   name=f"ri{h}{pi}")
                    nc.vector.reciprocal_approx_fast(
                        rinv[:, :qw], srow[:, :qw]
                    )
                    rbr = wp.tile([D, SLOTW], f32, tag="rbr", bufs=3,
                                  name=f"rr{h}{pi}")
                    nc.gpsimd.partition_broadcast(rbr[:, :qw], rinv[:, :qw])
                    nc.vector.tensor_mul(
                        ot[qr : qr + D, qc, qo : qo + qw],
                        stg[0:D, :qw],
                        rbr[:, :qw],
                    )

                    if h == H - 1 and pi == NPIECE - 1:
                        for j in range(NCH):
                            if qo <= j * P and j * P + CHS[j] <= qo + qw:
                                emit_proj(j, use_dve=(j % 2 == 0))

                def emit_proj(j, use_dve):
                    m = CHS[j]
                    yp = sps.tile([P, SLOTW], f32, tag="sc", name=f"yp{j}")
                    for c in range(3):
                        nc.tensor.matmul(
                            yp[:m, :C],
                            ot[:, c, j * P : j * P + m],
                            wproj[:, c, :],
                            start=(c == 0),
                            stop=(c == 2 and not HASB),
                        )
                    if HASB:
                        nc.tensor.matmul(
                            yp[:m, :C],
                            ones[:, :m],
                            bvec[:, :],
                            start=False,
                            stop=True,
                        )
                    ys = wp.tile([P, C], f32, tag="ys", name=f"ys{j}")
                    if use_dve:
                        nc.vector.tensor_copy(ys[:m, :], yp[:m, :C])
                    else:
                        nc.scalar.copy(ys[:m, :], yp[:m, :C])
                    # split per chunk across two hw queues (per-queue bw is
                    # ~23 GB/s; one 196KB chunk would be an ~8us drain) and
                    # dispatch from compute engines to spare the sync queue
                    hm = (m + 1) // 2
                    nc.gpsimd.dma_start(
                        out_d[j * P : j * P + hm, :], ys[0:hm, :]
                    )
                    nc.sync.dma_start(
                        out_d[j * P + hm : j * P + m, :], ys[hm:m, :]
                    )

                # prepass: PE warm-up dummies while the first DMAs land,
                # then the minimal qkv units for (h0, piece0) chunk 0
                dz = sps.tile([P, 512], f32, tag="sc", name="warmmm")
                for _ in range(2):
                    nc.tensor.matmul(
                        dz[:, :], ones[:, :], wrow[:, :],
                        start=True, stop=True,
                    )
                junk = pp.tile([P, 1], f32, tag="junk")
                nc.vector.tensor_copy(junk[:, :], dz[:, 0:1])
                _qkv_unit(0, 0, 512)()
                _qkv_unit(3, 0, 512)()
                _qkv_unit(0, 512, 512)()
                emit_scores_g(*passes[0], 0)
                _v_unit(0)()
                for idx in range(len(passes)):
                    emit_pass(idx)

    nc.finalize()
    return nc


def _prep_core_inputs(x_b, p_b, wqkvT, wprojT, bvec, ident):
    """Permute tokens kept-keys-first; build exp-bias and diag-fix tensors.
    Returns (in_map, perm)."""
    import ml_dtypes

    bf16 = ml_dtypes.bfloat16
    perm = np.argsort(-p_b, kind="stable")
    xT = np.ascontiguousarray(x_b[perm].T).astype(bf16)
    p_perm = p_b[perm].astype(np.float32)
    pad = NCH * P - N
    p_pad = np.concatenate([p_perm, np.zeros(pad, np.float32)])
    # bias_exp[r, i] = -30 * (1 - p[i*128 + r]) per key chunk
    bias = (NEG * (1.0 - p_pad)).reshape(NCH, P).T.copy()
    # dfix[:, i, :] = diag(240 * (1 - p_chunk_i)) as bf16
    dfix = np.zeros((P, NCH, P), np.float32)
    for i in range(NCH):
        chunk = p_pad[i * P : (i + 1) * P]
        np.fill_diagonal(dfix[:, i, :], DIAGV * (1.0 - chunk))
    return {
        "xT": xT,
        "wqkvT": wqkvT,
        "wprojT": wprojT,
        "bias_exp": np.ascontiguousarray(bias),
        "dfix": dfix.astype(bf16),
        "ident": ident,
        "bvec": bvec,
    }, perm


def _install_ntff_hook():
    """The container's antenv package lacks axon_hooks; recreate the NTFF
    profile hook (mirrors trn_agent_boot) so trace=True yields exec_time."""
    import types
    import ctypes
    import contextlib

    if "antenv.axon_hooks" in sys.modules:
        return
    so_path = "/opt/axon/libaxon_pjrt.so"
    mod = types.ModuleType("antenv.axon_hooks")
    state = {"hook": None}
    mod.set_axon_ntff_profile_hook = lambda h: state.__setitem__("hook", h)
    mod.get_axon_ntff_profile_hook = lambda: state["hook"]
    sys.modules["antenv.axon_hooks"] = mod

    try:
        lib = ctypes.CDLL(so_path)
    except OSError:
        return
    if not hasattr(lib, "axon_start_nrt_profile"):
        return
    lib.axon_start_nrt_profile.argtypes = [
        ctypes.POINTER(ctypes.c_int64),
        ctypes.c_size_t,
    ]
    lib.axon_start_nrt_profile.restype = ctypes.c_int64
    lib.axon_stop_nrt_profile.argtypes = [ctypes.c_char_p]
    lib.axon_stop_nrt_profile.restype = ctypes.c_int64

    @contextlib.contextmanager
    def _hook(output_dir, device_ids):
        import jax

        jax.devices()
        if device_ids:
            ids = (ctypes.c_int64 * len(device_ids))(*device_ids)
            rc = lib.axon_start_nrt_profile(ids, len(device_ids))
        else:
            rc = lib.axon_start_nrt_profile(None, 0)
        if rc != 0:
            raise RuntimeError(f"axon_start_nrt_profile rc={rc}")
        try:
            yield
        finally:
            n = lib.axon_stop_nrt_profile(str(output_dir).encode())
            print(f"profile: {n} file(s) written to {output_dir}", file=sys.stderr)

    state["hook"] = _hook


def kernel(x, vis_tube, w_qkv, w_proj, b_proj, _trace=False):
    from concourse.bass_utils import run_bass_kernel_spmd

    import ml_dtypes

    if _trace:
        _install_ntff_hook()

    bf16 = ml_dtypes.bfloat16
    x = np.asarray(x, np.float32)
    p = np.asarray(vis_tube, np.float32)[:, :, 0]
    keeps = (p > 0.5).sum(axis=1)  # kept keys per batch
    KC = max(1, int(-(-keeps.max() // P)))  # chunks containing kept keys
    BSTART = int(keeps.min() // P)  # first chunk containing a dropped key

    HASB = bool(np.any(np.asarray(b_proj)))
    key = (KC, BSTART, HASB)
    if _CACHE.get("key") != key:
        _CACHE["nc"] = _build_nc(KC, BSTART, HASB)
        _CACHE["key"] = key
    nc = _CACHE["nc"]

    wqkvT = np.ascontiguousarray(np.asarray(w_qkv).T).astype(bf16)
    wprojT = np.ascontiguousarray(np.asarray(w_proj).T).astype(bf16)
    bvec = np.asarray(b_proj).reshape(1, C).astype(np.float32).astype(bf16)
    ident = np.eye(P, dtype=np.float32).astype(bf16)
    in_maps, perms = [], []
    for b in range(B):
        im, perm = _prep_core_inputs(x[b], p[b], wqkvT, wprojT, bvec, ident)
        in_maps.append(im)
        perms.append(perm)
    res = run_bass_kernel_spmd(nc, in_maps, core_ids=list(range(B)), trace=_trace)
    out = np.empty((B, N, C), np.float32)
    for b in range(B):
        out[b][perms[b]] = np.asarray(res.results[b]["out"], np.float32)
    if _trace:
        _CACHE["last_result"] = res
    return out
